# Initial kernel scaffold
#
"""Trainium2 Bass kernel for AbsolutePositionEncoding (embedding lookup + broadcast).

Reference computation (x's values are irrelevant — only its shape matters):
    idx  = arange(2048) // 8           # rows 0..255 of the table, each repeated 8x
    rows = E[idx]                      # [2048, 256]
    out  = broadcast(rows, (64, 2048, 256))

Data-parallel over batch: each of the 8 cores produces an identical
[8, 2048, 256] f32 shard (16 MiB); the host concatenates to [64, 2048, 256].

Per-core device schedule (pure DMA problem; ~48.5-49.7 us measured incl.
fixed NEFF overheads, vs ~46 us bound = 38.6 us fabric-floor stream + ~5 us
immovable preamble + ~2.3 us final HBM write receipt):
  1. One 256 KiB DMA: table rows E[0:256] -> SBUF [128, 512]
     (partition p holds rows 2p, 2p+1 contiguously).
  2. Batch 0, first half, as a DRAM->DRAM copy reading E rows straight from
     HBM with a 0-stride repeat AP. It depends on NOTHING, so the SDMA
     engines roll from the input into it with no semaphore-receipt bubble
     (~2 us saved vs waiting for the table load to confirm). Meanwhile the
     DVE expands the table into `rows` [128, 4096] (partition p =
     E[2p] x8 ++ E[2p+1] x8 = output rows 16p..16p+15, 16 KiB contiguous).
  3. Batch 0, second half, from `rows` (8 KiB descriptors) as soon as the
     second-half copies land.
  4. Batches 1..7 as ONE broadcast-source DMA (0-stride batch dim) with
     16 KiB descriptors per (partition, batch) — peak DMA efficiency.

All output DMAs keep the full 128-partition shape: partition-subset or
strided-partition DMAs are ~2x slower (verified on HW and in the cost model).
Semaphores: one per dependency group — `then_inc(sem, 16)` is 16 independent
+1s from the SDMA engines, so waits on a shared counter would race.
"""

import numpy as np

import concourse.bass as bass
import concourse.mybir as mybir
from concourse.bass_utils import run_bass_kernel_spmd

BATCH = 64
SEQ = 2048
EDIM = 256
OBJ = 512
ATTR = 8
NCORES = 8
B_SH = BATCH // NCORES  # 8 batch elements per core
ROWS_USED = SEQ // ATTR  # 256 table rows actually used


def _build() -> bass.Bass:
    # Bass.__init__ registers four const-AP SBUF tensors via gpsimd.memset.
    # Those memsets are DMA-backed; the init barrier's gpsimd drain waits out
    # their completion receipts (~4.5 us measured). This kernel never reads
    # the const APs, so suppress the memsets during construction.
    try:
        cls = bass.BassEitherVectorEngine
        orig_memset = cls.memset

        class _FakeInst:
            def then_inc(self, *a, **k):
                return self

        cls.memset = lambda self, ap, constant: _FakeInst()
        try:
            return _build_graph()
        finally:
            cls.memset = orig_memset
    except AttributeError:
        return _build_graph()


def _build_graph() -> bass.Bass:
    nc = bass.Bass()
    e_ext = nc.declare_dram_parameter("e", [OBJ, EDIM], mybir.dt.float32, isOutput=False)
    out_ext = nc.declare_dram_parameter(
        "out", [B_SH, SEQ, EDIM], mybir.dt.float32, isOutput=True
    )

    # Bare sems (no context): the preamble clears the whole kernel sem range,
    # and skipping the context exit avoids a per-sem clear + barrier tail.
    in_sem = nc.alloc_semaphore("in_sem")
    out_sem = nc.alloc_semaphore("out_sem")
    cp_sem = nc.alloc_semaphore("cp_sem")

    with (
        nc.sbuf_tensor([128, 2 * EDIM], mybir.dt.float32) as table,
        nc.sbuf_tensor([128, 16 * EDIM], mybir.dt.float32) as rows,
        nc.Block(no_gpsimd_drain=True) as block,
    ):
        # [8, 2048, 256] -> [128 partitions, 8 batches, 4096 elems]:
        # partition p owns output rows 16p..16p+15 (16 KiB contiguous per batch)
        out_v = out_ext.rearrange("b (p n) e -> p b (n e)", p=128)

        @block.sync
        def _(sync: bass.BassEngine):
            src = e_ext[0:ROWS_USED, :].rearrange("(p k) e -> p (k e)", k=2)
            sync.dma_start(out=table[:], in_=src).then_inc(in_sem, 16)

            # batch 0 / rows 16p..16p+7: DRAM->DRAM from E rows 2p, repeat x8
            d0src = (
                e_ext[0:ROWS_USED:2, :].unsqueeze(1).broadcast_to([128, ATTR, EDIM])
            )
            sync.dma_start(
                out=out_v[:, 0, 0:2048].rearrange("p (r e) -> p r e", r=ATTR),
                in_=d0src,
            ).then_inc(out_sem, 16)

            # batch 0 / rows 16p+8..16p+15: from expanded rows (8 KiB descs)
            sync.wait_ge(cp_sem, 8)
            sync.dma_start(
                out=out_v[:, 0, 2048:4096], in_=rows[:, 2048:4096]
            ).then_inc(out_sem, 16)

            # batches 1..7 in one DMA (16 KiB descs, 0-stride batch source)
            sync.wait_ge(cp_sem, 16)
            b7 = rows[:].unsqueeze(1).broadcast_to([128, B_SH - 1, 16 * EDIM])
            sync.dma_start(out=out_v[:, 1:8, :], in_=b7).then_inc(out_sem, 16)
            sync.wait_ge(out_sem, 48)
            sync.wait_ge(in_sem, 16)

        @block.vector
        def _(vector: bass.BassEngine):
            vector.wait_ge(in_sem, 16)
            # second-half blocks first: batch 0's second half consumes them
            for j in list(range(8, 16)) + list(range(0, 8)):
                half = j // ATTR
                vector.tensor_copy(
                    rows[:, j * EDIM : (j + 1) * EDIM],
                    table[:, half * EDIM : (half + 1) * EDIM],
                ).then_inc(cp_sem, 1)

    return nc


_NC: bass.Bass | None = None


def kernel(x: np.ndarray, E_absolute_position: np.ndarray) -> np.ndarray:
    global _NC
    if _NC is None:
        _NC = _build()
    nc = _NC
    table = np.ascontiguousarray(np.asarray(E_absolute_position, dtype=np.float32))
    in_maps = [{"e": table} for _ in range(NCORES)]
    res = run_bass_kernel_spmd(nc, in_maps, core_ids=list(range(NCORES)))
    shards = [res.results[i]["out"] for i in range(NCORES)]
    return np.concatenate(shards, axis=0)



# revision 1
# speedup vs baseline: 1.1194x; 1.1194x over previous
"""Trainium2 Bass kernel for AbsolutePositionEncoding (embedding lookup + broadcast).

Reference computation (x's values are irrelevant — only its shape matters):
    idx  = arange(2048) // 8           # rows 0..255 of the table, each repeated 8x
    rows = E[idx]                      # [2048, 256]
    out  = broadcast(rows, (64, 2048, 256))

Data-parallel over batch: each of the 8 cores produces an identical
[8, 2048, 256] f32 shard (16 MiB); the host concatenates to [64, 2048, 256].

Per-core device schedule (pure DMA problem; ~48.5-49.7 us measured incl.
fixed NEFF overheads, vs ~46 us bound = 38.6 us fabric-floor stream + ~5 us
immovable preamble + ~2.3 us final HBM write receipt):
  1. One 256 KiB DMA: table rows E[0:256] -> SBUF [128, 512]
     (partition p holds rows 2p, 2p+1 contiguously).
  2. Batch 0, first half, as a DRAM->DRAM copy reading E rows straight from
     HBM with a 0-stride repeat AP. It depends on NOTHING, so the SDMA
     engines roll from the input into it with no semaphore-receipt bubble
     (~2 us saved vs waiting for the table load to confirm). Meanwhile the
     DVE expands the table into `rows` [128, 4096] (partition p =
     E[2p] x8 ++ E[2p+1] x8 = output rows 16p..16p+15, 16 KiB contiguous).
  3. Batch 0, second half, from `rows` (8 KiB descriptors) as soon as the
     second-half copies land.
  4. Batches 1..7 as ONE broadcast-source DMA (0-stride batch dim) with
     16 KiB descriptors per (partition, batch) — peak DMA efficiency.

All output DMAs keep the full 128-partition shape: partition-subset or
strided-partition DMAs are ~2x slower (verified on HW and in the cost model).
Semaphores: one per dependency group — `then_inc(sem, 16)` is 16 independent
+1s from the SDMA engines, so waits on a shared counter would race.
"""

import numpy as np

import concourse.bass as bass
import concourse.mybir as mybir
from concourse.bass_utils import run_bass_kernel_spmd

BATCH = 64
SEQ = 2048
EDIM = 256
OBJ = 512
ATTR = 8
NCORES = 8
B_SH = BATCH // NCORES  # 8 batch elements per core
ROWS_USED = SEQ // ATTR  # 256 table rows actually used


def _build() -> bass.Bass:
    # Bass.__init__ registers four const-AP SBUF tensors via gpsimd.memset.
    # Those memsets are DMA-backed; the init barrier's gpsimd drain waits out
    # their completion receipts (~4.5 us measured). This kernel never reads
    # the const APs, so suppress the memsets during construction.
    try:
        cls = bass.BassEitherVectorEngine
        orig_memset = cls.memset

        class _FakeInst:
            def then_inc(self, *a, **k):
                return self

        cls.memset = lambda self, ap, constant: _FakeInst()
        try:
            return _build_graph()
        finally:
            cls.memset = orig_memset
    except AttributeError:
        return _build_graph()


def _build_graph() -> bass.Bass:
    nc = bass.Bass()
    e_ext = nc.declare_dram_parameter("e", [OBJ, EDIM], mybir.dt.float32, isOutput=False)
    out_ext = nc.declare_dram_parameter(
        "out", [B_SH, SEQ, EDIM], mybir.dt.float32, isOutput=True
    )

    # Bare sems (no context): the preamble clears the whole kernel sem range,
    # and skipping the context exit avoids a per-sem clear + barrier tail.
    in_sem = nc.alloc_semaphore("in_sem")
    out_sem = nc.alloc_semaphore("out_sem")
    cp_sem = nc.alloc_semaphore("cp_sem")

    with (
        nc.sbuf_tensor([128, 2 * EDIM], mybir.dt.float32) as table,
        nc.sbuf_tensor([128, 16 * EDIM], mybir.dt.float32) as rows,
        nc.Block(no_gpsimd_drain=True) as block,
    ):
        # [8, 2048, 256] -> [128 partitions, 8 batches, 4096 elems]:
        # partition p owns output rows 16p..16p+15 (16 KiB contiguous per batch)
        out_v = out_ext.rearrange("b (p n) e -> p b (n e)", p=128)

        @block.sync
        def _(sync: bass.BassEngine):
            src = e_ext[0:ROWS_USED, :].rearrange("(p k) e -> p (k e)", k=2)
            sync.dma_start(out=table[:], in_=src).then_inc(in_sem, 16)

            # batch 0 / rows 16p..16p+7: DRAM->DRAM from E rows 2p, repeat x8
            d0src = (
                e_ext[0:ROWS_USED:2, :].unsqueeze(1).broadcast_to([128, ATTR, EDIM])
            )
            sync.dma_start(
                out=out_v[:, 0, 0:2048].rearrange("p (r e) -> p r e", r=ATTR),
                in_=d0src,
            ).then_inc(out_sem, 16)

            # batch 0 / rows 16p+8..16p+15: from expanded rows (8 KiB descs)
            sync.wait_ge(cp_sem, 8)
            sync.dma_start(
                out=out_v[:, 0, 2048:4096], in_=rows[:, 2048:4096]
            ).then_inc(out_sem, 16)

            # batches 1..7 in one DMA (16 KiB descs, 0-stride batch source)
            sync.wait_ge(cp_sem, 16)
            b7 = rows[:].unsqueeze(1).broadcast_to([128, B_SH - 1, 16 * EDIM])
            sync.dma_start(out=out_v[:, 1:8, :], in_=b7).then_inc(out_sem, 16)
            sync.wait_ge(out_sem, 48)
            sync.wait_ge(in_sem, 16)

        @block.vector
        def _(vector: bass.BassEngine):
            vector.wait_ge(in_sem, 16)
            # second-half blocks first: batch 0's second half consumes them
            for j in list(range(8, 16)) + list(range(0, 8)):
                half = j // ATTR
                vector.tensor_copy(
                    rows[:, j * EDIM : (j + 1) * EDIM],
                    table[:, half * EDIM : (half + 1) * EDIM],
                ).then_inc(cp_sem, 1)

    return nc


_NC: bass.Bass | None = None


def kernel(x: np.ndarray, E_absolute_position: np.ndarray) -> np.ndarray:
    global _NC
    if _NC is None:
        _NC = _build()
    nc = _NC
    table = np.ascontiguousarray(np.asarray(E_absolute_position, dtype=np.float32))
    in_maps = [{"e": table} for _ in range(NCORES)]
    res = run_bass_kernel_spmd(nc, in_maps, core_ids=list(range(NCORES)))
    shards = [res.results[i]["out"] for i in range(NCORES)]
    return np.concatenate(shards, axis=0)

